# revision 25
# baseline (speedup 1.0000x reference)
"""Trainium2 Bass kernel for CycleBalanceLoss (v4).

loss = ALPHA * mean_b |sum_l adj[b, a_l, a_{l+1}]|        (a = argmax_n logits)
     + (1-ALPHA) * mean_{b,l} (ln sum_n exp(x) - x_target)

Sharding: pure data parallel over batch B=64 across 8 cores (BPC=8).
Host sums the per-core partials ([128,2] per core).

Key layout/algorithm choices (baseline was 53us: GPSIMD queue saturated by
16 indirect DMAs; DVE did 2 full scans/batch plus per-batch small ops):

- Host ROTATES each logits row by its target index:
      lgrot[b, l, n] = lg[b, l, (n + t[b,l]) % N]
  so the target logit sits at column 0 of every row and is extracted with
  one strided DVE copy instead of 8 indirect DMAs.  The rotated argmax r
  relates to the true argmax by a = (r + t) mod N.
- The mod is made free by uploading a doubled adjacency ADJ2[2048, 2048]
  (ADJ2[u, v] = adj[u & 1023, v & 1023]); the gather offset is
  2048*(r+t)_l + (r+t)_{l+1} with (r+t) in [0, 2046].
- argmax: reduce_max into col 0 of an [L,8] tile + FIND_INDEX8 (cols 1-7
  are don't-care match slots).  Batch 0 is N-split into two halves so the
  DVE pipeline starts ~1.3us earlier (reduce_max per half + tiny max).
- pair offsets via PE matmul against PM = 2048*I + subdiag(1) (the subdiag
  is the l+1 partition shift; engines cannot read partition-offset APs),
  +(2048*t_l + t_{l+1}) host constant folded into the PSUM->SBUF copy.
- one indirect DMA per batch (adj gather, 128 offsets; pad row 127 reads
  offset 0 and is excluded from the balance sum).
- per-batch balance sums go into a [1,8] PSUM row vector right after each
  gather; the tail is one |.|-sum reduce + a [128,2] result DMA, with the
  final scalar reduction on the host.
- consts (PM, TP) are loaded via the otherwise-idle GPSIMD SWDGE queue;
  no activation-table churn: Exp table loads once up front, Ln's load
  happens in the ACT queue's idle window before the tail needs it.
"""

import numpy as np

B, L, N = 64, 128, 1024
N2 = 2 * N
NH = N // 2
NCORES = 8
BPC = B // NCORES
ALPHA = 0.7

_CACHE = {}


def _build():
    import concourse.bacc as bacc
    import concourse.tile as tile
    from concourse import bass, mybir

    f32 = mybir.dt.float32
    fp16 = mybir.dt.float16
    i32 = mybir.dt.int32
    u16 = mybir.dt.uint16
    AF = mybir.ActivationFunctionType
    Alu = mybir.AluOpType
    AX = mybir.AxisListType

    nc = bacc.Bacc(
        "TRN2",
        target_bir_lowering=False,
        debug=False,
        num_devices=NCORES,
    )

    logits = nc.dram_tensor("logits", [BPC, L, N], f32, kind="ExternalInput")
    tp2 = nc.dram_tensor("tp2", [L, BPC], f32, kind="ExternalInput")
    pairm = nc.dram_tensor("pairm", [128, 128], f32, kind="ExternalInput")
    ident = nc.dram_tensor("ident", [128, 128], f32, kind="ExternalInput")
    adjt = [
        nc.dram_tensor(f"adj{b}", [N2 * N2, 1], f32, kind="ExternalInput")
        for b in range(BPC)
    ]
    out = nc.dram_tensor("out", [L, 2], f32, kind="ExternalOutput")

    logits_ap = logits.ap()

    with tile.TileContext(nc) as tc:
        with (
            tc.tile_pool(name="acc", bufs=1) as accp,
            tc.tile_pool(name="wk", bufs=2) as ep,
            tc.tile_pool(name="psum", bufs=2, space="PSUM") as pp,
        ):
            cp = ep
            XB = accp.tile([L, BPC * N], f32)
            ones = accp.tile([L, 1], f32)
            TP = accp.tile([L, BPC], f32)    # host const: 2048*t_l + t_{l+1}
            PM = accp.tile([128, 128], f32)  # host const: 2048*I + subdiag(1)
            ID = accp.tile([128, 128], f32)  # host const: identity
            S = accp.tile([L, BPC], f32)     # per-row sum(exp)
            SB2 = accp.tile([L, 1], f32)     # batch-0 second-half accum
            XT = accp.tile([L, BPC], f32)    # target logits (col 0 of slabs)
            M8 = accp.tile([L, 8], fp16)     # match slots (col0 = row max)
            IDX = accp.tile([L, BPC], f32)   # rotated argmax as f32
            PAIR = accp.tile([L, BPC], i32)  # gather offsets
            W = accp.tile([L, BPC], f32)     # gathered path weights
            LSE = accp.tile([L, BPC], f32)
            NLLt = accp.tile([L, BPC], f32)
            R = accp.tile([L, 2], f32)

            # stream batch 0 in two N-halves (earlier DVE start), rest whole
            nc.sync.dma_start(XB[:, 0:NH], logits_ap[0][:, 0:NH])
            nc.sync.dma_start(XB[:, NH:N], logits_ap[0][:, NH:N])
            for b in range(1, BPC):
                nc.sync.dma_start(XB[:, b * N : (b + 1) * N], logits_ap[b])
            # consts via the (idle until first gather) GPSIMD SWDGE queue
            nc.gpsimd.dma_start(TP[:], tp2.ap())
            nc.gpsimd.dma_start(PM[:], pairm.ap())
            nc.gpsimd.dma_start(ID[:], ident.ap())

            nc.vector.memset(ones[:], 1.0)

            PAIRP = pp.tile([L, BPC], f32)  # pair offsets staging in PSUM

            def tail_ops(b, i8):
                # pair_l = 2048*r_l + r_{l+1} + (2048*t_l + t_{l+1}); the
                # partition shift is PM's subdiag, the host constant TP is
                # folded in with a second accumulating matmul so the only
                # per-batch DVE op is the u16->f32 index cast.
                nc.vector.tensor_copy(IDX[:, b : b + 1], i8[:, 0:1])
                nc.tensor.matmul(
                    out=PAIRP[:, b : b + 1], lhsT=PM[:], rhs=IDX[:, b : b + 1],
                    start=True, stop=False,
                )
                nc.tensor.matmul(
                    out=PAIRP[:, b : b + 1], lhsT=ID[:], rhs=TP[:, b : b + 1],
                    start=False, stop=True,
                )

            # ---- batch 0 (split halves) ----
            E0 = ep.tile([L, N], fp16, tag="E")
            nc.scalar.activation(
                E0[:, 0:NH], XB[:, 0:NH], AF.Exp, accum_out=S[:, 0:1]
            )
            M8A = ep.tile([L, 8], fp16, tag="m8a")
            nc.vector.max(M8A[:], E0[:, 0:NH])
            nc.scalar.activation(
                E0[:, NH:N], XB[:, NH:N], AF.Exp, accum_out=SB2[:]
            )
            nc.vector.max(M8[:], E0[:, NH:N])
            nc.vector.tensor_tensor(
                M8[:, 0:1], M8[:, 0:1], M8A[:, 0:1], op=Alu.max
            )
            i8 = cp.tile([L, 8], u16, tag="i8")
            nc.vector.max_index(i8[:], M8[:], E0[:])
            tail_ops(0, i8)

            # ---- batches 1..7 ----
            for b in range(1, BPC):
                Xb = XB[:, b * N : (b + 1) * N]
                E = ep.tile([L, N], fp16, tag="E")
                nc.scalar.activation(E[:], Xb, AF.Exp, accum_out=S[:, b : b + 1])
                nc.vector.max(M8[:], E[:])
                i8 = cp.tile([L, 8], u16, tag="i8")
                nc.vector.max_index(i8[:], M8[:], E[:])
                tail_ops(b, i8)

            # PSUM->SBUF pair copies on ACT (table-free Copy, after the exp
            # stream so the ACT queue never stalls an exp), then gathers
            for b in range(BPC):
                nc.scalar.activation(
                    PAIR[:, b : b + 1], PAIRP[:, b : b + 1], AF.Copy
                )
                nc.gpsimd.indirect_dma_start(
                    out=W[:, b : b + 1],
                    out_offset=None,
                    in_=adjt[b].ap(),
                    in_offset=bass.IndirectOffsetOnAxis(
                        ap=PAIR[:, b : b + 1], axis=0
                    ),
                )

            # balance per-batch path sums in one PE matmul after all gathers
            ps1 = pp.tile([1, BPC], f32)
            nc.tensor.matmul(
                out=ps1[:],
                lhsT=ones[0 : L - 1, :],
                rhs=W[0 : L - 1, :],
                start=True,
                stop=True,
            )

            # fix up batch-0 split accumulators + extract target logits
            nc.vector.tensor_add(S[:, 0:1], S[:, 0:1], SB2[:])
            nc.vector.tensor_copy(
                XT[:], XB[:].rearrange("p (b n) -> p b n", b=BPC)[:, :, 0:1]
            )

            # cross-entropy partial: R[:,0] = sum_b (ln S - x_t)
            nc.scalar.activation(LSE[:], S[:], AF.Ln)
            nc.vector.tensor_sub(NLLt[:], LSE[:], XT[:])
            nc.vector.reduce_sum(R[:, 0:1], NLLt[:], axis=AX.X)

            # balance partial: R[0,1] = sum_b |path sum_b|
            nc.vector.tensor_reduce(
                R[0:1, 1:2], ps1[:], axis=AX.X, op=Alu.add,
                apply_absolute_value=True,
            )
            nc.scalar.dma_start(out.ap(), R[:])

    nc.compile()
    return nc


def _get_nc():
    if "nc" not in _CACHE:
        _CACHE["nc"] = _build()
    return _CACHE["nc"]


def _pairm():
    if "pairm" not in _CACHE:
        ls = np.arange(128)
        pm = float(N2) * (ls[:, None] == ls[None, :]).astype(np.float32) + (
            ls[:, None] == ls[None, :] + 1
        ).astype(np.float32)
        _CACHE["pairm"] = pm
    return _CACHE["pairm"]


def make_in_maps(path_logits, target_paths, adj_matrix):
    """Shard + repack full inputs into per-core in_maps (host-side only)."""
    ar = np.arange(N, dtype=np.int64)
    pm = _pairm()
    in_maps = []
    for c in range(NCORES):
        sl = slice(c * BPC, (c + 1) * BPC)
        lg = np.asarray(path_logits[sl], dtype=np.float32)
        t = np.asarray(target_paths[sl], dtype=np.int64)  # [BPC, L]
        # rotate each row so the target logit is at column 0
        rot = (ar[None, None, :] + t[:, :, None]) % N
        lgrot = np.ascontiguousarray(np.take_along_axis(lg, rot, axis=2))
        # host constant: 2048*t_l + t_{l+1} (last row: 2048*t_127)
        tp = float(N2) * t.astype(np.float64)
        tp[:, : L - 1] += t[:, 1:].astype(np.float64)
        ads = {}
        for b in range(BPC):
            a = np.asarray(adj_matrix[c * BPC + b], dtype=np.float32)
            ads[f"adj{b}"] = np.ascontiguousarray(
                np.tile(a, (2, 2))
            ).reshape(N2 * N2, 1)
        in_maps.append(
            {
                "logits": lgrot,
                "tp2": np.ascontiguousarray(tp.T.astype(np.float32)),
                "pairm": pm,
                "ident": np.eye(128, dtype=np.float32),
                **ads,
            }
        )
    return in_maps


def kernel(**inputs):
    from concourse import bass_utils

    nc = _get_nc()
    in_maps = make_in_maps(
        inputs["path_logits"], inputs["target_paths"], inputs["adj_matrix"]
    )
    res = bass_utils.run_bass_kernel_spmd(nc, in_maps, core_ids=list(range(NCORES)))
    w_nll = np.float32((1.0 - ALPHA) / (B * L))
    w_bal = np.float32(ALPHA / B)
    total = np.float32(0.0)
    for r in res.results:
        ro = np.asarray(r["out"], dtype=np.float32)
        total = total + w_nll * np.float32(ro[:, 0].sum()) + w_bal * np.float32(
            ro[0, 1]
        )
    return np.asarray(total, dtype=np.float32)


# revision 28
# speedup vs baseline: 1.0896x; 1.0896x over previous
"""Trainium2 Bass kernel for CycleBalanceLoss (v4).

loss = ALPHA * mean_b |sum_l adj[b, a_l, a_{l+1}]|        (a = argmax_n logits)
     + (1-ALPHA) * mean_{b,l} (ln sum_n exp(x) - x_target)

Sharding: pure data parallel over batch B=64 across 8 cores (BPC=8).
Host sums the per-core partials ([128,2] per core).

Key layout/algorithm choices (baseline was 53us: GPSIMD queue saturated by
16 indirect DMAs; DVE did 2 full scans/batch plus per-batch small ops):

- Host ROTATES each logits row by its target index:
      lgrot[b, l, n] = lg[b, l, (n + t[b,l]) % N]
  so the target logit sits at column 0 of every row and is extracted with
  one strided DVE copy instead of 8 indirect DMAs.  The rotated argmax r
  relates to the true argmax by a = (r + t) mod N.
- The mod is made free by uploading a doubled adjacency ADJ2[2048, 2048]
  (ADJ2[u, v] = adj[u & 1023, v & 1023]); the gather offset is
  2048*(r+t)_l + (r+t)_{l+1} with (r+t) in [0, 2046].
- argmax: reduce_max into col 0 of an [L,8] tile + FIND_INDEX8 (cols 1-7
  are don't-care match slots).  Batch 0 is N-split into two halves so the
  DVE pipeline starts ~1.3us earlier (reduce_max per half + tiny max).
- pair offsets via PE matmul against PM = 2048*I + subdiag(1) (the subdiag
  is the l+1 partition shift; engines cannot read partition-offset APs),
  +(2048*t_l + t_{l+1}) host constant folded into the PSUM->SBUF copy.
- one indirect DMA per batch (adj gather, 128 offsets; pad row 127 reads
  offset 0 and is excluded from the balance sum).
- per-batch balance sums go into a [1,8] PSUM row vector right after each
  gather; the tail is one |.|-sum reduce + a [128,2] result DMA, with the
  final scalar reduction on the host.
- consts (PM, TP) are loaded via the otherwise-idle GPSIMD SWDGE queue;
  no activation-table churn: Exp table loads once up front, Ln's load
  happens in the ACT queue's idle window before the tail needs it.
"""

import numpy as np

B, L, N = 64, 128, 1024
N2 = 2 * N
NH = N // 2
NCORES = 8
BPC = B // NCORES
ALPHA = 0.7

_CACHE = {}


def _build():
    import concourse.bacc as bacc
    import concourse.tile as tile
    from concourse import bass, mybir

    f32 = mybir.dt.float32
    fp16 = mybir.dt.float16
    i32 = mybir.dt.int32
    u16 = mybir.dt.uint16
    AF = mybir.ActivationFunctionType
    Alu = mybir.AluOpType
    AX = mybir.AxisListType

    nc = bacc.Bacc(
        "TRN2",
        target_bir_lowering=False,
        debug=False,
        num_devices=NCORES,
    )

    logits = nc.dram_tensor("logits", [BPC, L, N], f32, kind="ExternalInput")
    tp2 = nc.dram_tensor("tp2", [L, BPC], f32, kind="ExternalInput")
    pairm = nc.dram_tensor("pairm", [128, 128], f32, kind="ExternalInput")
    adjt = [
        nc.dram_tensor(f"adj{b}", [N2 * N2, 1], f32, kind="ExternalInput")
        for b in range(BPC)
    ]
    out = nc.dram_tensor("out", [L, 2], f32, kind="ExternalOutput")

    logits_ap = logits.ap()

    with tile.TileContext(nc) as tc:
        with (
            tc.tile_pool(name="acc", bufs=1) as accp,
            tc.tile_pool(name="wk", bufs=3) as ep,
            tc.tile_pool(name="psum", bufs=2, space="PSUM") as pp,
        ):
            cp = ep
            XB = accp.tile([L, BPC * N], f32)
            ones = accp.tile([L, 1], f32)
            TP = accp.tile([L, BPC], f32)    # host const: 2048*t_l + t_{l+1}
            PM = accp.tile([128, 128], f32)  # host const: 2048*I + subdiag(1)
            S = accp.tile([L, BPC], f32)     # per-row sum(exp)
            SB2 = accp.tile([L, 1], f32)     # batch-0 second-half accum
            XT = accp.tile([L, BPC], f32)    # target logits (col 0 of slabs)
            M8 = accp.tile([L, 8], fp16)     # match slots (col0 = row max)
            IDX = accp.tile([L, BPC], f32)   # rotated argmax as f32
            PAIR = accp.tile([L, BPC], i32)  # gather offsets
            W = accp.tile([L, BPC], f32)     # gathered path weights
            LSE = accp.tile([L, BPC], f32)
            NLLt = accp.tile([L, BPC], f32)
            R = accp.tile([L, 2], f32)

            # stream batch 0 in two N-halves (earlier DVE start), rest whole
            nc.sync.dma_start(XB[:, 0:NH], logits_ap[0][:, 0:NH])
            nc.sync.dma_start(XB[:, NH:N], logits_ap[0][:, NH:N])
            for b in range(1, BPC):
                nc.sync.dma_start(XB[:, b * N : (b + 1) * N], logits_ap[b])
            # consts via the (idle until first gather) GPSIMD SWDGE queue
            nc.gpsimd.dma_start(TP[:], tp2.ap())
            nc.gpsimd.dma_start(PM[:], pairm.ap())

            nc.vector.memset(ones[:], 1.0)

            def tail_ops(b, i8):
                # pair_l = 2048*r_l + r_{l+1} via PE (partition shift is the
                # subdiag of PM; engines cannot read partition-offset APs),
                # then +(2048*t_l + t_{l+1}) host constant during PSUM copy
                nc.vector.tensor_copy(IDX[:, b : b + 1], i8[:, 0:1])
                pairp = pp.tile([L, 1], f32)
                nc.tensor.matmul(
                    out=pairp[:], lhsT=PM[:], rhs=IDX[:, b : b + 1],
                    start=True, stop=True,
                )
                nc.vector.scalar_tensor_tensor(
                    PAIR[:, b : b + 1],
                    pairp[:],
                    1.0,
                    TP[:, b : b + 1],
                    op0=Alu.mult,
                    op1=Alu.add,
                )
                nc.gpsimd.indirect_dma_start(
                    out=W[:, b : b + 1],
                    out_offset=None,
                    in_=adjt[b].ap(),
                    in_offset=bass.IndirectOffsetOnAxis(
                        ap=PAIR[:, b : b + 1], axis=0
                    ),
                )

            # ---- batch 0 (split halves) ----
            E0 = ep.tile([L, N], fp16, tag="E")
            nc.scalar.activation(
                E0[:, 0:NH], XB[:, 0:NH], AF.Exp, accum_out=S[:, 0:1]
            )
            M8A = ep.tile([L, 8], fp16, tag="m8a")
            nc.vector.max(M8A[:], E0[:, 0:NH])
            nc.scalar.activation(
                E0[:, NH:N], XB[:, NH:N], AF.Exp, accum_out=SB2[:]
            )
            nc.vector.max(M8[:], E0[:, NH:N])
            nc.vector.tensor_tensor(
                M8[:, 0:1], M8[:, 0:1], M8A[:, 0:1], op=Alu.max
            )
            i8 = cp.tile([L, 8], u16, tag="i8")
            nc.vector.max_index(i8[:], M8[:], E0[:])
            prev = (0, i8)

            # ---- batches 1..7, software-pipelined: batch b-1's
            # index->pair->gather chain is emitted after batch b's max/find
            # so the DVE queue keeps MAX8/FIND back-to-back ----
            for b in range(1, BPC):
                Xb = XB[:, b * N : (b + 1) * N]
                E = ep.tile([L, N], fp16, tag="E")
                nc.scalar.activation(E[:], Xb, AF.Exp, accum_out=S[:, b : b + 1])
                nc.vector.max(M8[:], E[:])
                i8 = cp.tile([L, 8], u16, tag="i8")
                nc.vector.max_index(i8[:], M8[:], E[:])
                tail_ops(*prev)
                prev = (b, i8)
            tail_ops(*prev)

            # balance per-batch path sums in one PE matmul after all gathers
            ps1 = pp.tile([1, BPC], f32)
            nc.tensor.matmul(
                out=ps1[:],
                lhsT=ones[0 : L - 1, :],
                rhs=W[0 : L - 1, :],
                start=True,
                stop=True,
            )

            # fix up batch-0 split accumulators + extract target logits
            nc.vector.tensor_add(S[:, 0:1], S[:, 0:1], SB2[:])
            nc.vector.tensor_copy(
                XT[:], XB[:].rearrange("p (b n) -> p b n", b=BPC)[:, :, 0:1]
            )

            # cross-entropy partial: R[:,0] = sum_b (ln S - x_t)
            nc.scalar.activation(LSE[:], S[:], AF.Ln)
            nc.vector.tensor_sub(NLLt[:], LSE[:], XT[:])
            nc.vector.reduce_sum(R[:, 0:1], NLLt[:], axis=AX.X)

            # balance partial: R[0,1] = sum_b |path sum_b|
            nc.vector.tensor_reduce(
                R[0:1, 1:2], ps1[:], axis=AX.X, op=Alu.add,
                apply_absolute_value=True,
            )
            nc.scalar.dma_start(out.ap(), R[:])

    nc.compile()
    return nc


def _get_nc():
    if "nc" not in _CACHE:
        _CACHE["nc"] = _build()
    return _CACHE["nc"]


def _pairm():
    if "pairm" not in _CACHE:
        ls = np.arange(128)
        pm = float(N2) * (ls[:, None] == ls[None, :]).astype(np.float32) + (
            ls[:, None] == ls[None, :] + 1
        ).astype(np.float32)
        _CACHE["pairm"] = pm
    return _CACHE["pairm"]


def make_in_maps(path_logits, target_paths, adj_matrix):
    """Shard + repack full inputs into per-core in_maps (host-side only)."""
    ar = np.arange(N, dtype=np.int64)
    pm = _pairm()
    in_maps = []
    for c in range(NCORES):
        sl = slice(c * BPC, (c + 1) * BPC)
        lg = np.asarray(path_logits[sl], dtype=np.float32)
        t = np.asarray(target_paths[sl], dtype=np.int64)  # [BPC, L]
        # rotate each row so the target logit is at column 0
        rot = (ar[None, None, :] + t[:, :, None]) % N
        lgrot = np.ascontiguousarray(np.take_along_axis(lg, rot, axis=2))
        # host constant: 2048*t_l + t_{l+1} (last row: 2048*t_127)
        tp = float(N2) * t.astype(np.float64)
        tp[:, : L - 1] += t[:, 1:].astype(np.float64)
        ads = {}
        for b in range(BPC):
            a = np.asarray(adj_matrix[c * BPC + b], dtype=np.float32)
            ads[f"adj{b}"] = np.ascontiguousarray(
                np.tile(a, (2, 2))
            ).reshape(N2 * N2, 1)
        in_maps.append(
            {
                "logits": lgrot,
                "tp2": np.ascontiguousarray(tp.T.astype(np.float32)),
                "pairm": pm,
                **ads,
            }
        )
    return in_maps


def kernel(**inputs):
    from concourse import bass_utils

    nc = _get_nc()
    in_maps = make_in_maps(
        inputs["path_logits"], inputs["target_paths"], inputs["adj_matrix"]
    )
    res = bass_utils.run_bass_kernel_spmd(nc, in_maps, core_ids=list(range(NCORES)))
    w_nll = np.float32((1.0 - ALPHA) / (B * L))
    w_bal = np.float32(ALPHA / B)
    total = np.float32(0.0)
    for r in res.results:
        ro = np.asarray(r["out"], dtype=np.float32)
        total = total + w_nll * np.float32(ro[:, 0].sum()) + w_bal * np.float32(
            ro[0, 1]
        )
    return np.asarray(total, dtype=np.float32)


# revision 29
# speedup vs baseline: 1.2788x; 1.1737x over previous
"""Trainium2 Bass kernel for CycleBalanceLoss (v4).

loss = ALPHA * mean_b |sum_l adj[b, a_l, a_{l+1}]|        (a = argmax_n logits)
     + (1-ALPHA) * mean_{b,l} (ln sum_n exp(x) - x_target)

Sharding: pure data parallel over batch B=64 across 8 cores (BPC=8).
Host sums the per-core partials ([128,2] per core).

Key layout/algorithm choices (baseline was 53us: GPSIMD queue saturated by
16 indirect DMAs; DVE did 2 full scans/batch plus per-batch small ops):

- Host ROTATES each logits row by its target index:
      lgrot[b, l, n] = lg[b, l, (n + t[b,l]) % N]
  so the target logit sits at column 0 of every row and is extracted with
  one strided DVE copy instead of 8 indirect DMAs.  The rotated argmax r
  relates to the true argmax by a = (r + t) mod N.
- The mod is made free by uploading a doubled adjacency ADJ2[2048, 2048]
  (ADJ2[u, v] = adj[u & 1023, v & 1023]); the gather offset is
  2048*(r+t)_l + (r+t)_{l+1} with (r+t) in [0, 2046].
- argmax: reduce_max into col 0 of an [L,8] tile + FIND_INDEX8 (cols 1-7
  are don't-care match slots).  Batch 0 is N-split into two halves so the
  DVE pipeline starts ~1.3us earlier (reduce_max per half + tiny max).
- pair offsets via PE matmul against PM = 2048*I + subdiag(1) (the subdiag
  is the l+1 partition shift; engines cannot read partition-offset APs),
  +(2048*t_l + t_{l+1}) host constant folded into the PSUM->SBUF copy.
- one indirect DMA per batch (adj gather, 128 offsets; pad row 127 reads
  offset 0 and is excluded from the balance sum).
- per-batch balance sums go into a [1,8] PSUM row vector right after each
  gather; the tail is one |.|-sum reduce + a [128,2] result DMA, with the
  final scalar reduction on the host.
- consts (PM, TP) are loaded via the otherwise-idle GPSIMD SWDGE queue;
  no activation-table churn: Exp table loads once up front, Ln's load
  happens in the ACT queue's idle window before the tail needs it.
"""

import numpy as np

B, L, N = 64, 128, 1024
N2 = 2 * N
NH = N // 2
NCORES = 8
BPC = B // NCORES
ALPHA = 0.7

_CACHE = {}


def _build():
    import concourse.bacc as bacc
    import concourse.tile as tile
    from concourse import bass, mybir

    f32 = mybir.dt.float32
    fp16 = mybir.dt.float16
    i32 = mybir.dt.int32
    u16 = mybir.dt.uint16
    AF = mybir.ActivationFunctionType
    Alu = mybir.AluOpType
    AX = mybir.AxisListType

    nc = bacc.Bacc(
        "TRN2",
        target_bir_lowering=False,
        debug=False,
        num_devices=NCORES,
    )

    logits = nc.dram_tensor("logits", [BPC, L, N], f32, kind="ExternalInput")
    tp2 = nc.dram_tensor("tp2", [L, BPC], f32, kind="ExternalInput")
    pairm = nc.dram_tensor("pairm", [128, 128], f32, kind="ExternalInput")
    adjt = [
        nc.dram_tensor(f"adj{b}", [N2 * N2, 1], f32, kind="ExternalInput")
        for b in range(BPC)
    ]
    out = nc.dram_tensor("out", [L, 2], f32, kind="ExternalOutput")

    logits_ap = logits.ap()

    with tile.TileContext(nc) as tc:
        with (
            tc.tile_pool(name="acc", bufs=1) as accp,
            tc.tile_pool(name="wk", bufs=3) as ep,
            tc.tile_pool(name="psum", bufs=2, space="PSUM") as pp,
        ):
            cp = ep
            XB = accp.tile([L, BPC * N], f32)
            ones = accp.tile([L, 1], f32)
            TP = accp.tile([L, BPC], f32)    # host const: 2048*t_l + t_{l+1}
            PM = accp.tile([128, 128], f32)  # host const: 2048*I + subdiag(1)
            S = accp.tile([L, BPC], f32)     # per-row sum(exp)
            SB2 = accp.tile([L, 1], f32)     # batch-0 second-half accum
            XT = accp.tile([L, BPC], f32)    # target logits (col 0 of slabs)
            IDX = accp.tile([L, BPC], f32)   # rotated argmax as f32
            PAIR = accp.tile([L, BPC], i32)  # gather offsets
            W = accp.tile([L, BPC], f32)     # gathered path weights
            LSE = accp.tile([L, BPC], f32)
            NLLt = accp.tile([L, BPC], f32)
            R = accp.tile([L, 2], f32)

            # stream batch 0 in two N-halves (earlier DVE start), rest whole
            nc.sync.dma_start(XB[:, 0:NH], logits_ap[0][:, 0:NH])
            nc.sync.dma_start(XB[:, NH:N], logits_ap[0][:, NH:N])
            for b in range(1, BPC):
                nc.sync.dma_start(XB[:, b * N : (b + 1) * N], logits_ap[b])
            # consts via the (idle until first gather) GPSIMD SWDGE queue
            nc.gpsimd.dma_start(TP[:], tp2.ap())
            nc.gpsimd.dma_start(PM[:], pairm.ap())

            nc.vector.memset(ones[:], 1.0)

            def tail_ops(b, i8):
                # pair_l = 2048*r_l + r_{l+1} via PE (partition shift is the
                # subdiag of PM; engines cannot read partition-offset APs),
                # then +(2048*t_l + t_{l+1}) host constant during PSUM copy
                nc.vector.tensor_copy(IDX[:, b : b + 1], i8[:, 0:1])
                pairp = pp.tile([L, 1], f32)
                nc.tensor.matmul(
                    out=pairp[:], lhsT=PM[:], rhs=IDX[:, b : b + 1],
                    start=True, stop=True,
                )
                nc.vector.scalar_tensor_tensor(
                    PAIR[:, b : b + 1],
                    pairp[:],
                    1.0,
                    TP[:, b : b + 1],
                    op0=Alu.mult,
                    op1=Alu.add,
                )
                nc.gpsimd.indirect_dma_start(
                    out=W[:, b : b + 1],
                    out_offset=None,
                    in_=adjt[b].ap(),
                    in_offset=bass.IndirectOffsetOnAxis(
                        ap=PAIR[:, b : b + 1], axis=0
                    ),
                )

            # ---- batch 0 (split halves) ----
            E0 = ep.tile([L, N], fp16, tag="E")
            nc.scalar.activation(
                E0[:, 0:NH], XB[:, 0:NH], AF.Exp, accum_out=S[:, 0:1]
            )
            M8A = ep.tile([L, 8], fp16, tag="m8a")
            nc.vector.max(M8A[:], E0[:, 0:NH])
            nc.scalar.activation(
                E0[:, NH:N], XB[:, NH:N], AF.Exp, accum_out=SB2[:]
            )
            M8 = ep.tile([L, 8], fp16, tag="m8")
            nc.vector.max(M8[:], E0[:, NH:N])
            nc.vector.tensor_tensor(
                M8[:, 0:1], M8[:, 0:1], M8A[:, 0:1], op=Alu.max
            )
            i8 = cp.tile([L, 8], u16, tag="i8")
            nc.vector.max_index(i8[:], M8[:], E0[:])
            prev = (0, i8)

            # ---- batches 1..7, software-pipelined: batch b-1's
            # index->pair->gather chain is emitted after batch b's max/find
            # so the DVE queue keeps MAX8/FIND back-to-back ----
            for b in range(1, BPC):
                Xb = XB[:, b * N : (b + 1) * N]
                E = ep.tile([L, N], fp16, tag="E")
                nc.scalar.activation(E[:], Xb, AF.Exp, accum_out=S[:, b : b + 1])
                M8 = ep.tile([L, 8], fp16, tag="m8")
                nc.vector.max(M8[:], E[:])
                i8 = cp.tile([L, 8], u16, tag="i8")
                nc.vector.max_index(i8[:], M8[:], E[:])
                tail_ops(*prev)
                prev = (b, i8)
            tail_ops(*prev)

            # balance per-batch path sums in one PE matmul after all gathers
            ps1 = pp.tile([1, BPC], f32)
            nc.tensor.matmul(
                out=ps1[:],
                lhsT=ones[0 : L - 1, :],
                rhs=W[0 : L - 1, :],
                start=True,
                stop=True,
            )

            # fix up batch-0 split accumulators + extract target logits
            nc.vector.tensor_add(S[:, 0:1], S[:, 0:1], SB2[:])
            nc.vector.tensor_copy(
                XT[:], XB[:].rearrange("p (b n) -> p b n", b=BPC)[:, :, 0:1]
            )

            # cross-entropy partial: R[:,0] = sum_b (ln S - x_t)
            nc.scalar.activation(LSE[:], S[:], AF.Ln)
            nc.vector.tensor_sub(NLLt[:], LSE[:], XT[:])
            nc.vector.reduce_sum(R[:, 0:1], NLLt[:], axis=AX.X)

            # balance partial: R[0,1] = sum_b |path sum_b|
            nc.vector.tensor_reduce(
                R[0:1, 1:2], ps1[:], axis=AX.X, op=Alu.add,
                apply_absolute_value=True,
            )
            nc.scalar.dma_start(out.ap(), R[:])

    nc.compile()
    return nc


def _get_nc():
    if "nc" not in _CACHE:
        _CACHE["nc"] = _build()
    return _CACHE["nc"]


def _pairm():
    if "pairm" not in _CACHE:
        ls = np.arange(128)
        pm = float(N2) * (ls[:, None] == ls[None, :]).astype(np.float32) + (
            ls[:, None] == ls[None, :] + 1
        ).astype(np.float32)
        _CACHE["pairm"] = pm
    return _CACHE["pairm"]


def make_in_maps(path_logits, target_paths, adj_matrix):
    """Shard + repack full inputs into per-core in_maps (host-side only)."""
    ar = np.arange(N, dtype=np.int64)
    pm = _pairm()
    in_maps = []
    for c in range(NCORES):
        sl = slice(c * BPC, (c + 1) * BPC)
        lg = np.asarray(path_logits[sl], dtype=np.float32)
        t = np.asarray(target_paths[sl], dtype=np.int64)  # [BPC, L]
        # rotate each row so the target logit is at column 0
        rot = (ar[None, None, :] + t[:, :, None]) % N
        lgrot = np.ascontiguousarray(np.take_along_axis(lg, rot, axis=2))
        # host constant: 2048*t_l + t_{l+1} (last row: 2048*t_127)
        tp = float(N2) * t.astype(np.float64)
        tp[:, : L - 1] += t[:, 1:].astype(np.float64)
        ads = {}
        for b in range(BPC):
            a = np.asarray(adj_matrix[c * BPC + b], dtype=np.float32)
            ads[f"adj{b}"] = np.ascontiguousarray(
                np.tile(a, (2, 2))
            ).reshape(N2 * N2, 1)
        in_maps.append(
            {
                "logits": lgrot,
                "tp2": np.ascontiguousarray(tp.T.astype(np.float32)),
                "pairm": pm,
                **ads,
            }
        )
    return in_maps


def kernel(**inputs):
    from concourse import bass_utils

    nc = _get_nc()
    in_maps = make_in_maps(
        inputs["path_logits"], inputs["target_paths"], inputs["adj_matrix"]
    )
    res = bass_utils.run_bass_kernel_spmd(nc, in_maps, core_ids=list(range(NCORES)))
    w_nll = np.float32((1.0 - ALPHA) / (B * L))
    w_bal = np.float32(ALPHA / B)
    total = np.float32(0.0)
    for r in res.results:
        ro = np.asarray(r["out"], dtype=np.float32)
        total = total + w_nll * np.float32(ro[:, 0].sum()) + w_bal * np.float32(
            ro[0, 1]
        )
    return np.asarray(total, dtype=np.float32)
